# revision 37
# baseline (speedup 1.0000x reference)
"""LocalWindowAttention (block-causal) Trainium2 kernel, 8 NeuronCores.

Sharding: tensor-parallel over heads. Core c owns head-columns
[c*128, (c+1)*128) of the D=1024 hidden dim (2 heads x head_dim 64):
computes its Q/K/V projections (transposed layout), block-causal
attention for its 2 heads, AllGathers the normalized attention outputs
(two chunk-pair AGs), applies the full Wo for its 128 output rows of
final^T [1024, 2048]. Host reassembles.

Schedule: the exp stream on the Activation engine (~39us total) is the
critical path of the middle; the AllGather completion is set by the
slowest core's trigger time (launch skew + CC-init dominate the tail),
so every us shaved off the trigger times pays 1:1:
  - pass 1 computes only Q/K cols 0-1023 (x-DMA-arrival-paced), so the
    first scores/exps issue at ~31us. V passes, V^T PE-transposes, and
    pass-B Q/K all run later from the resident x tiles as PE filler
    interleaved UNDER the exp stream (one shared 1-bank psum + a
    transpose bank), including c2's first four score/exp items.
  - phase 3 drains the AV backlog between fresh score/exp pairs so the
    exp stream never stalls behind a long AV block in the PE FIFO.
    AG {c0,c1} triggers at ~71us (was 78), AG {c2,c3} at ~96us
    (was 105), with per-chunk HWDGE staging of the AG inputs.
  - softmax denominators ride as ones-columns in the attn@V operand
    (cols 64-127 of vn -> 64 denominator replicas on psum partitions
    64-127); normalize is a cross-partition-base DVE copy + 
    reciprocal_approx_fast at full lane width + multiply (the original
    single-lane reciprocal cost 3.3us per call).
  - output projection per AG group; gather prefetches alternate the two
    HWDGE queues; PSUM->SBUF y copies on ACT (idle after exps) so the
    scheduler cannot block normalize muls behind the outproj chain.
  - Wk is pre-scaled by HD^-0.5 on the host so exp needs no scale.
  - bf16 operands on-chip (PSUM stays f32); host casts inputs to bf16.
Measured: 134-146us across runs (baseline 170-199us); rel err 5.2e-3.
"""

import numpy as np
import ml_dtypes

import concourse.bacc as bacc
import concourse.tile as tile
from concourse import mybir
from concourse.bass_utils import run_bass_kernel_spmd
from concourse.masks import make_identity

B, T, D = 1, 2048, 1024
H, HD, W = 16, 64, 128
N_CORES = 8
HS = D // N_CORES        # 128 head-columns per core (2 heads)
HPC = H // N_CORES       # heads per core
QW = 512                 # query-chunk width (free dim of S^T tiles)
NQ = T // QW             # 4 query chunks
NK = T // W              # 16 key chunks of 128
ND = D // 128            # 8 contraction chunks over D
SCALE = HD ** -0.5

F32 = mybir.dt.float32
BF16 = mybir.dt.bfloat16
Exp = mybir.ActivationFunctionType.Exp

_compiled = {}


def _build():
    nc = bacc.Bacc("TRN2", target_bir_lowering=False, debug=False,
                   num_devices=N_CORES)
    xT_ap = nc.dram_tensor("xT", [D, T], BF16, kind="ExternalInput").ap()
    wq_ap = nc.dram_tensor("wq", [D, HS], BF16, kind="ExternalInput").ap()
    wk_ap = nc.dram_tensor("wk", [D, HS], BF16, kind="ExternalInput").ap()
    wv_ap = nc.dram_tensor("wv", [D, HS], BF16, kind="ExternalInput").ap()
    wo_ap = nc.dram_tensor("wo", [D, HS], BF16, kind="ExternalInput").ap()
    y_ap = nc.dram_tensor("y", [HS, T], F32, kind="ExternalOutput").ap()

    with tile.TileContext(nc) as tc:
        _body(tc, xT_ap, wq_ap, wk_ap, wv_ap, wo_ap, y_ap)
    nc.compile()
    return nc


def _body(tc, xT_ap, wq_ap, wk_ap, wv_ap, wo_ap, y_ap):
    nc = tc.nc
    from contextlib import ExitStack
    with ExitStack() as ctx:
        singles = ctx.enter_context(tc.tile_pool(name="singles", bufs=1))
        work = ctx.enter_context(tc.tile_pool(name="work", bufs=4))
        es_pool = ctx.enter_context(tc.tile_pool(name="es_pool", bufs=16))
        dram = ctx.enter_context(tc.tile_pool(name="dram", bufs=1, space="DRAM"))

        # ---- warmup collective first: absorbs CC init + launch skew ------
        warm_in = dram.tile([128, 8], F32, name="warm_in")
        warm_out = dram.tile([N_CORES, 128, 8], F32, addr_space="Shared",
                             name="warm_out")
        nc.gpsimd.collective_compute(
            "AllGather", mybir.AluOpType.bypass,
            replica_groups=[list(range(N_CORES))],
            ins=[warm_in.opt()], outs=[warm_out.opt()])

        # ---- weight DMAs -------------------------------------------------
        wq = singles.tile([128, ND, HS], BF16, tag="wq")
        wk = singles.tile([128, ND, HS], BF16, tag="wk")
        wv = singles.tile([128, ND, HS], BF16, tag="wv")
        wo = singles.tile([128, ND, HS], BF16, tag="wo")
        nc.sync.dma_start(out=wq[:], in_=wq_ap.rearrange("(c p) m -> p c m", p=128))
        nc.scalar.dma_start(out=wk[:], in_=wk_ap.rearrange("(c p) m -> p c m", p=128))
        nc.gpsimd.dma_start(out=wv[:], in_=wv_ap.rearrange("(c p) m -> p c m", p=128))

        qT = singles.tile([128, T], BF16, tag="qT")
        kT = singles.tile([128, T], BF16, tag="kT")
        vT = singles.tile([128, T], BF16, tag="vT")
        # vn[h]: per-head [key, hd | ones] operand for attn@V; cols HD..127
        # are softmax-denominator ones columns (the AV matmul then leaves 64
        # denominator replicas on psum partitions 64-127, which normalize
        # consumes with a cross-base DVE copy). memset whole tile to 1.0;
        # transposes overwrite cols 0:HD.
        vn = [singles.tile([128, NK, 128], BF16, tag=f"vn{h}", name=f"vn{h}")
              for h in range(HPC)]
        for h in range(HPC):
            nc.vector.memset(vn[h][:], 1.0)
        outT = singles.tile([128, T], BF16, tag="outT")

        qdma = [nc.sync, nc.scalar, nc.gpsimd]
        xts = []

        # ---- phase 1: Q/K for cols 0..1023 only, x DMAs interleaved ------
        # (V and all of pass B run later from the resident x tiles as PE
        # filler under the exp stream; dropping them here lets the first
        # scores issue ~5us earlier)
        with tc.tile_pool(name="pp_qk", bufs=1, space="PSUM") as pp_qk:
            ps_q = pp_qk.tile([128, 2, QW], F32, tag="q", name="ps_q")
            ps_k = pp_qk.tile([128, 2, QW], F32, tag="k", name="ps_k")
            XQ = [None, 0, 1, 2, 0, 1, 2, 0]
            for d in range(ND):
                xt = singles.tile([128, T], BF16, tag=f"x{d}", name=f"xt{d}")
                if d == 0:
                    nc.sync.dma_start(out=xt[0:64, :], in_=xT_ap[0:64, :])
                    nc.scalar.dma_start(out=xt[64:128, :], in_=xT_ap[64:128, :])
                else:
                    qdma[XQ[d]].dma_start(out=xt[:],
                                          in_=xT_ap[d * 128:(d + 1) * 128, :])
                xts.append(xt)
                f = (d == 0)
                l = (d == ND - 1)
                for ps, w_ in ((ps_q, wq), (ps_k, wk)):
                    for sub in range(2):
                        cs = slice(sub * QW, (sub + 1) * QW)
                        nc.tensor.matmul(ps[:, sub, :], w_[:, d, :],
                                         xts[d][:, cs], start=f, stop=l)
            nc.gpsimd.dma_start(out=wo[:],
                                in_=wo_ap.rearrange("(c p) m -> p c m", p=128))
            # identity for PE transposes (emitted after the DMAs so it never
            # delays their issue)
            ident_f32 = singles.tile([128, 128], F32, tag="ident_f32")
            make_identity(nc, ident_f32)
            ident = singles.tile([128, 128], BF16, tag="ident")
            nc.vector.tensor_copy(ident[:], ident_f32[:])
            # copies: kT on DVE and qT on ACT run concurrently so the first
            # score matmul (needs both) issues earlier
            nc.vector.tensor_copy(kT[:, 0:2 * QW], ps_k[:])
            nc.scalar.activation(out=qT[:, 0:2 * QW], in_=ps_q[:],
                                 func=mybir.ActivationFunctionType.Copy)

        # ---- attention machinery -----------------------------------------
        PAIRS = [(0, 2), (2, 2)]
        ag_in = [dram.tile([HS, n * QW], BF16, name=f"ag_in{j}")
                 for j, (s, n) in enumerate(PAIRS)]
        ag_out = [dram.tile([N_CORES, HS, n * QW], BF16, addr_space="Shared",
                            name=f"ag_out{j}") for j, (s, n) in enumerate(PAIRS)]

        def s_exp(t, tk, pa):
            qs = max(0, (tk - 4 * t) * W)
            ps_s = pa.tile([128, 2, QW], F32, tag="s", name="ps_s")
            for h in range(HPC):
                hrows = slice(h * HD, (h + 1) * HD)
                nc.tensor.matmul(
                    ps_s[:, h, qs:],
                    kT[hrows, tk * W:(tk + 1) * W],
                    qT[hrows, t * QW + qs:(t + 1) * QW],
                    start=True, stop=True)
            e = es_pool.tile([128, 2, QW], BF16, tag="es", name="es")
            nc.scalar.activation(out=e[:, :, qs:], in_=ps_s[:, :, qs:],
                                 func=Exp)
            return e

        ps_os = {}

        def av(t, tk, e, po):
            qs = max(0, (tk - 4 * t) * W)
            n_tk = 4 * t + 4
            if tk == 0:
                ps_os[t] = po.tile([128, 2, QW], F32, tag="o", name="ps_o")
            ps_o = ps_os[t]
            for h in range(HPC):
                nc.tensor.matmul(ps_o[:, h, qs:], vn[h][:, tk, :],
                                 e[:, h, qs:],
                                 start=(tk == 0), stop=(tk == n_tk - 1))
            if tk == n_tk - 1:
                _normalize(t, ps_o)

        def _normalize(t, ps_o):
            cols = slice(t * QW, (t + 1) * QW)
            # denominator replicas at psum partitions 64-127: cross-base DVE
            # copy to partitions 0-63, reciprocal full-width, multiply.
            dcp = work.tile([HD, 2, QW], F32, tag="dcp", name="dcp")
            nc.vector.tensor_copy(dcp[:], ps_o[HD:2 * HD, :, :])
            rbc = work.tile([HD, 2, QW], F32, tag="rbc", name="rbc")
            nc.vector.reciprocal_approx_fast(rbc[:], dcp[:])
            for h in range(HPC):
                hrows = slice(h * HD, (h + 1) * HD)
                nc.vector.tensor_mul(outT[hrows, cols],
                                     ps_o[:HD, h, :], rbc[:, h, :])
            # stage this chunk's columns into its group's AG input now
            # (HWDGE; sync queue is idle here), so the group-closing chunk
            # only waits on its own 256KB before the trigger
            j = [j for j, (s, n) in enumerate(PAIRS)
                 if s <= t < s + n][0]
            s, n = PAIRS[j]
            nc.sync.dma_start(out=ag_in[j][:, (t - s) * QW:(t - s + 1) * QW],
                              in_=outT[:, cols])
            if t == s + n - 1:
                nc.gpsimd.collective_compute(
                    "AllGather", mybir.AluOpType.bypass,
                    replica_groups=[list(range(N_CORES))],
                    ins=[ag_in[j].opt()], outs=[ag_out[j].opt()])

        # ---- phase 2: scores+exps c0+c1; V-pass-A + transposes 0-7,
        # pass-B Q/K, V-pass-B + transposes 8-15 as interleaved PE filler
        with tc.tile_pool(name="pa", bufs=2, space="PSUM") as pa, \
             tc.tile_pool(name="util", bufs=1, space="PSUM") as util:
            fill = []
            st = {}

            def f_mm(w_, d, cs, f, l):
                def op():
                    if f:
                        st["t"] = util.tile([128, QW], F32, tag="pb",
                                            name="ps_pb")
                    nc.tensor.matmul(st["t"][:], w_[:, d, :], xts[d][:, cs],
                                     start=f, stop=l)
                return op

            def f_copy(dst, cs):
                def op():
                    nc.vector.tensor_copy(dst[:, cs], st["t"][:])
                return op

            def f_tr(tk):
                def op():
                    ps_t = util.tile([128, W], BF16, tag="t", name="ps_t")
                    nc.tensor.transpose(ps_t[:], vT[:, tk * W:(tk + 1) * W],
                                        ident[:])
                    for h in range(HPC):
                        nc.vector.tensor_copy(vn[h][:, tk, 0:HD],
                                              ps_t[:, h * HD:(h + 1) * HD])
                return op

            def f_sub(w_, dst, sub, trs):
                cs = slice(sub * QW, (sub + 1) * QW)
                ops = [f_mm(w_, d, cs, d == 0, d == ND - 1) for d in range(ND)]
                ops.append(f_copy(dst, cs))
                ops.extend(f_tr(tk) for tk in trs)
                return ops

            # V-A (+T0-7) first (c0/c1 AVs need vn 0-7 at ~46), then pass-B
            # Q/K (c2 scores need qB0/kB0 at ~46), then V-B (+T8-15)
            fill += f_sub(wv, vT, 0, range(0, 4))
            fill += f_sub(wv, vT, 1, range(4, 8))
            fill += f_sub(wq, qT, 2, ())
            fill += f_sub(wk, kT, 2, ())
            fill += f_sub(wq, qT, 3, ())
            fill += f_sub(wk, kT, 3, ())
            fill += f_sub(wv, vT, 2, range(8, 12))
            fill += f_sub(wv, vT, 3, range(12, 16))

            # c0+c1 scores/exps under fill[0:44] (V-A + transposes 0-7 +
            # Q-B), then c2's first four items (their q columns land with
            # the Q-B copy) under the remaining fill
            items1 = [(t, tk) for t in range(2) for tk in range(4 * t + 4)]
            items1 += [(2, tk) for tk in range(4)]
            es1 = {}
            fi = 0
            for idx, it in enumerate(items1):
                es1[it] = s_exp(*it, pa)
                goal = (idx + 1) * len(fill) // len(items1)
                while fi < goal:
                    fill[fi]()
                    fi += 1
            while fi < len(fill):
                fill[fi]()
                fi += 1

        # ---- phase 3: AVs for c0+c1 (AG j0), then pipelined c2+c3 --------
        with tc.tile_pool(name="pa2", bufs=2, space="PSUM") as pa2, \
             tc.tile_pool(name="po", bufs=2, space="PSUM") as po:
            # backlog: AVs for everything exp'd in phase 2, interleaved
            # between fresh c2 score/exp pairs so the exp stream never
            # waits behind a long AV block in the PE FIFO
            backlog = [(it, es1[it]) for it in items1]
            items2 = [(2, tk) for tk in range(4, 12)]
            items2 += [(3, tk) for tk in range(16)]
            pend = []
            BPI = 4
            for it in items2:
                pend.append((it, s_exp(*it, pa2)))
                for _ in range(BPI):
                    if backlog:
                        (bt, btk), be = backlog.pop(0)
                        av(bt, btk, be, po)
                while not backlog and len(pend) > 2:
                    (pt_, ptk), pe_ = pend.pop(0)
                    av(pt_, ptk, pe_, po)
            while backlog:
                (bt, btk), be = backlog.pop(0)
                av(bt, btk, be, po)
            for (pt_, ptk), pe_ in pend:
                av(pt_, ptk, pe_, po)

        # ---- output projection on gathered activations -------------------
        gt_pool = ctx.enter_context(tc.tile_pool(name="gt_pool", bufs=1))
        with tc.tile_pool(name="py", bufs=1, space="PSUM") as py:
            for j, (s, n) in enumerate(PAIRS):
                gts = []
                for c in range(N_CORES):
                    g = gt_pool.tile([128, n * QW], BF16,
                                     tag=f"g{j}_{c}", name=f"g{j}_{c}")
                    gq = nc.sync if c % 2 == 0 else nc.scalar
                    gq.dma_start(out=g[:], in_=ag_out[j][c])
                    gts.append(g)
                ps_y = [py.tile([128, QW], F32, tag=f"y{i}", name=f"ps_y{i}")
                        for i in range(n)]
                for c in range(N_CORES):
                    for i in range(n):
                        nc.tensor.matmul(ps_y[i][:], wo[:, c, :],
                                         gts[c][:, i * QW:(i + 1) * QW],
                                         start=(c == 0), stop=(c == N_CORES - 1))
                for i in range(n):
                    t = s + i
                    cols = slice(t * QW, (t + 1) * QW)
                    # cy copies on ACT (idle after the exps; keeps DVE free
                    # for the normalize muls)
                    cy = work.tile([128, QW], F32, tag=f"cy{j}_{i}")
                    nc.scalar.activation(
                        out=cy[:], in_=ps_y[i][:],
                        func=mybir.ActivationFunctionType.Copy)
                    (nc.sync if i % 2 == 0 else nc.scalar).dma_start(
                        out=y_ap[:, cols], in_=cy[:])


def make_in_maps(x, Wq, Wk, Wv, Wo):
    bf = ml_dtypes.bfloat16
    xT = np.ascontiguousarray(np.asarray(x).reshape(T, D).T).astype(bf)
    Wk_s = np.asarray(Wk) * SCALE  # fold softmax scale into K projection
    in_maps = []
    for c in range(N_CORES):
        hs = slice(c * HS, (c + 1) * HS)
        in_maps.append({
            "xT": xT,
            "wq": np.ascontiguousarray(np.asarray(Wq)[:, hs]).astype(bf),
            "wk": np.ascontiguousarray(Wk_s[:, hs]).astype(bf),
            "wv": np.ascontiguousarray(np.asarray(Wv)[:, hs]).astype(bf),
            "wo": np.ascontiguousarray(np.asarray(Wo)[:, hs]).astype(bf),
        })
    return in_maps


def kernel(x, Wq, Wk, Wv, Wo):
    if "nc" not in _compiled:
        _compiled["nc"] = _build()
    nc = _compiled["nc"]

    in_maps = make_in_maps(x, Wq, Wk, Wv, Wo)
    res = run_bass_kernel_spmd(nc, in_maps, list(range(N_CORES)))
    finalT = np.concatenate([res.results[c]["y"] for c in range(N_CORES)], axis=0)
    return np.ascontiguousarray(finalT.T).reshape(B, T, D)
